# revision 1
# baseline (speedup 1.0000x reference)
"""DenseCapsuleLayer forward on 8 Trainium2 NeuronCores.

Strategy: shard the in_num (i) axis 8 ways (256 capsules/core) so each core
reads only its 16 MiB slice of W once. Dynamic routing is local in i except
the s-reduction, which is a 128 KiB AllReduce per iteration.

Per-core layout: everything lives in partition space p = 32*j + b where
i_local = 4*q + j (q in [0,64)), b in [0,32).
  u_hat (bf16): U1[32*j + b, ((q*32 + o)*32 + v)]
PE produces u_hat via block-diagonal "quad" matmuls (4 capsules per matmul,
K = 4 i's x 16 d = 64) and also accumulates s0 = sum_i u_hat directly.
s-passes: DVE mult (c * u_hat) then PE matmuls against a 0/1 selection
matrix accumulate over (q, j) in PSUM. a-passes: DVE mult + tensor_reduce
over v. softmax over o is free-dim-local.
"""
import sys
import numpy as np

sys.path.insert(0, "/opt/trn_rl_repo")

import concourse.bass as bass
import concourse.mybir as mybir
import concourse.tile as tile
from concourse.bass_utils import run_bass_kernel_spmd

F32 = mybir.dt.float32
BF16 = mybir.dt.bfloat16
ALU = mybir.AluOpType
ACTF = mybir.ActivationFunctionType
AXX = mybir.AxisListType.X

NCORES = 8
B, IN, D = 32, 2048, 16
O, V = 32, 32
IL = IN // NCORES          # 256 local input capsules
NQ = IL // 4               # 64 quads
NCH = IL // 8              # 32 W chunks of [128, 1024]
OV = O * V                 # 1024
EPS = 1e-8

_CACHE = {}


class _PhaseAOnly(Exception):
    pass


def split_multi_waits(nc, max_waits=1):
    """walrus CoreV3 rejects instructions carrying several semaphore waits;
    move extras onto same-engine NoOps inserted just before."""
    ctr = [0]

    def fresh_nop(engine, wait):
        ctr[0] += 1
        nop = mybir.InstNoOp(
            name=f"wsplit_nop_{ctr[0]}", ins=[], outs=[],
            sync_info=mybir.SyncInfo(on_wait=[wait], on_update=[]),
        )
        nop.engine = engine
        return nop

    for fn in [nc.main_func]:
        for bb in fn.blocks:
            insts = bb.instructions
            i = 0
            while i < len(insts):
                inst = insts[i]
                si = getattr(inst, "sync_info", None)
                if si is not None and si.on_wait and len(si.on_wait) > max_waits:
                    extra = list(si.on_wait[max_waits:])
                    si.on_wait[:] = list(si.on_wait[:max_waits])
                    nops = [fresh_nop(inst.engine, w) for w in extra]
                    for n in nops:
                        nc.register_instruction(n, overwrite=True)
                    insts[i:i] = nops
                    i += len(nops)
                i += 1
    return nc


def _bcast_inner(ap, n):
    """Append a stride-0 innermost dim of size n to an AP view."""
    return bass.AP(ap.tensor, ap.offset, list(ap.ap) + [[0, n]])


def _build_nc(do_ar=True, do_apass=True, do_smult=True, phase_a_only=False, repeat=1):
    nc = bass.Bass("TRN2", target_bir_lowering=False, debug=False,
                   num_devices=NCORES)

    wa = nc.dram_tensor("wa", [NCH, 128, OV], F32, kind="ExternalInput")
    xq = nc.dram_tensor("xq", [NCH, 128, 256], F32, kind="ExternalInput")
    x2 = nc.dram_tensor("x2", [NCH, 128, 32], F32, kind="ExternalInput")
    sel = nc.dram_tensor("sel", [128, 32], BF16, kind="ExternalInput")
    y = nc.dram_tensor("y", [B, OV], F32, kind="ExternalOutput")

    ar_in = nc.dram_tensor("ar_in", [B, OV], F32)
    ar_out = nc.dram_tensor("ar_out", [B, OV], F32, addr_space="Shared")
    wr_d = nc.dram_tensor("wr_d", [B, OV], BF16)

    with tile.TileContext(nc) as tc:
        with (
            tc.tile_pool(name="u1", bufs=1) as u1_pool,
            tc.tile_pool(name="wtile", bufs=2) as w_pool,
            tc.tile_pool(name="xqtile", bufs=2) as xq_pool,
            tc.tile_pool(name="x2t", bufs=2) as x2_pool,
            tc.tile_pool(name="tmp", bufs=2) as tmp_pool,
            tc.tile_pool(name="big", bufs=1) as big_pool,
            tc.tile_pool(name="small", bufs=1) as small_pool,
            tc.tile_pool(name="psq", bufs=3, space="PSUM") as psq_pool,
            tc.tile_pool(name="pss", bufs=1, space="PSUM") as pss_pool,
        ):
            U1 = u1_pool.tile([128, NQ * OV], BF16)        # 128 KiB/part
            sel_t = small_pool.tile([128, 32], BF16, tag="sel")
            nc.sync.dma_start(sel_t[:], sel.ap())

            a0f = big_pool.tile([128, NQ * O], F32, tag="a0")   # L1 logits
            a1f = big_pool.tile([128, NQ * O], F32, tag="a1")   # a_t / L2
            ct = big_pool.tile([128, NQ * O], BF16, tag="ct")   # c_t bf16
            mx = small_pool.tile([128, NQ], F32, tag="mx")
            zz = small_pool.tile([128, NQ], F32, tag="zz")
            wrep = small_pool.tile([128, OV], BF16, tag="wrep")
            sg = small_pool.tile([32, OV], F32, tag="sg")
            sqs = small_pool.tile([32, OV], F32, tag="sqs")
            sq = small_pool.tile([32, O], F32, tag="sq")
            c1t = small_pool.tile([32, O], F32, tag="c1t")
            c2t = small_pool.tile([32, O], F32, tag="c2t")
            wv = small_pool.tile([32, OV], F32, tag="wv")
            wvb = small_pool.tile([32, OV], BF16, tag="wvb")
            epsb = small_pool.tile([32, 1], F32, tag="epsb")
            zb = small_pool.tile([128, 1], F32, tag="zb")
            nc.vector.memset(epsb[:], EPS)
            nc.vector.memset(zb[:], 0.0)

            # ---------- Phase A: u_hat production + s0 ----------
            s0ps = pss_pool.tile([32, OV], F32, tag="s0")
            for c in range(NCH):
                wt = w_pool.tile([128, OV], F32)
                xqt = xq_pool.tile([128, 256], F32)
                x2t = x2_pool.tile([128, 32], F32)
                nc.sync.dma_start(wt[:], wa.ap()[c])
                nc.sync.dma_start(xqt[:], xq.ap()[c])
                nc.sync.dma_start(x2t[:], x2.ap()[c])
                for h in range(2):
                    nc.tensor.matmul(
                        s0ps[:, h * 512:(h + 1) * 512],
                        x2t[:],
                        wt[:, h * 512:(h + 1) * 512],
                        start=(c == 0), stop=(c == NCH - 1),
                    )
                for jj in range(2):
                    q = 2 * c + jj
                    qp = psq_pool.tile([128, OV], F32)
                    lhsT = xqt[64 * jj:64 * (jj + 1), 128 * jj:128 * (jj + 1)]
                    for h in range(2):
                        nc.tensor.matmul(
                            qp[:, h * 512:(h + 1) * 512],
                            lhsT,
                            wt[64 * jj:64 * (jj + 1), h * 512:(h + 1) * 512],
                            start=True, stop=True,
                        )
                    dst = U1[:, q * OV:(q + 1) * OV]
                    nc.vector.tensor_copy(dst[:, :512], qp[:, :512])
                    nc.scalar.copy(dst[:, 512:], qp[:, 512:])

            # ---------- helpers ----------
            def allreduce_s(src):
                nc.sync.dma_start(ar_in.ap(), src)
                if do_ar:
                    nc.gpsimd.collective_compute(
                        "AllReduce", ALU.add,
                        replica_groups=[list(range(NCORES))],
                        ins=[ar_in.ap()], outs=[ar_out.ap()],
                    )
                    nc.sync.dma_start(sg[:], ar_out.ap())
                else:
                    nc.sync.dma_start(sg[:], ar_in.ap())

            def squash_to_w(scale):
                """wv = squash(sg*scale); wrep = bf16 replica on 128 parts."""
                sgv = sg[:].rearrange("p (o v) -> p o v", o=O)
                nc.vector.tensor_tensor(
                    sqs[:].rearrange("p (o v) -> p o v", o=O),
                    sgv, sgv, op=ALU.mult)
                nc.vector.tensor_reduce(
                    sq[:], sqs[:].rearrange("p (o v) -> p o v", o=O),
                    axis=AXX, op=ALU.add)
                if scale != 1.0:
                    nc.vector.tensor_scalar_mul(sq[:], sq[:], scale * scale)
                nc.vector.tensor_scalar_add(c1t[:], sq[:], 1.0)
                nc.vector.reciprocal(c1t[:], c1t[:])
                nc.scalar.activation(c2t[:], sq[:], ACTF.Sqrt, bias=epsb[:])
                nc.vector.reciprocal(c2t[:], c2t[:])
                nc.vector.tensor_tensor(c1t[:], c1t[:], sq[:], op=ALU.mult)
                nc.vector.tensor_tensor(c1t[:], c1t[:], c2t[:], op=ALU.mult)
                if scale != 1.0:
                    nc.vector.tensor_scalar_mul(c1t[:], c1t[:], scale)
                nc.vector.tensor_tensor(
                    wv[:].rearrange("p (o v) -> p o v", o=O), sgv,
                    _bcast_inner(c1t[:], V), op=ALU.mult)
                nc.vector.tensor_copy(wvb[:], wv[:])
                nc.sync.dma_start(wr_d.ap(), wvb[:])
                for j in range(4):
                    nc.sync.dma_start(wrep[32 * j:32 * (j + 1), :], wr_d.ap())

            def a_pass(dst):
                """dst[p,(q,o)] = sum_v u_hat * wrep."""
                if not do_apass:
                    nc.vector.memset(dst[:], 0.01)
                    return
                for qb in range(NQ // 4):
                    t = tmp_pool.tile([128, 4 * OV], BF16, tag="tp")
                    for qq in range(4):
                        q = qb * 4 + qq
                        nc.vector.tensor_tensor(
                            t[:, qq * OV:(qq + 1) * OV],
                            U1[:, q * OV:(q + 1) * OV],
                            wrep[:], op=ALU.mult)
                    # pairwise bf16 folds over v (2x DVE mode), then f32 reduce
                    tv = t[:].rearrange("p (q o v) -> p q o v", q=4, o=O)
                    nc.vector.tensor_tensor(
                        tv[:, :, :, 0:16], tv[:, :, :, 0:16],
                        tv[:, :, :, 16:32], op=ALU.add)
                    nc.vector.tensor_tensor(
                        tv[:, :, :, 0:8], tv[:, :, :, 0:8],
                        tv[:, :, :, 8:16], op=ALU.add)
                    nc.vector.tensor_reduce(
                        dst[:, qb * 4 * O:(qb + 1) * 4 * O]
                        .rearrange("p (q o) -> p q o", q=4),
                        tv[:, :, :, 0:8],
                        axis=AXX, op=ALU.add)

            def softmax(logits, scratch):
                """ct (bf16) = softmax over o of logits [p,(q,o)]."""
                lv = logits[:].rearrange("p (q o) -> p q o", q=NQ)
                sv = scratch[:].rearrange("p (q o) -> p q o", q=NQ)
                nc.vector.tensor_reduce(mx[:], lv, axis=AXX, op=ALU.max)
                nc.vector.tensor_tensor(
                    sv, lv, _bcast_inner(mx[:], O), op=ALU.subtract)
                nc.scalar.activation(scratch[:], scratch[:], ACTF.Exp, bias=zb[:])
                nc.vector.tensor_reduce(zz[:], sv, axis=AXX, op=ALU.add)
                nc.vector.reciprocal(zz[:], zz[:])
                nc.vector.tensor_tensor(
                    ct[:].rearrange("p (q o) -> p q o", q=NQ), sv,
                    _bcast_inner(zz[:], O), op=ALU.mult)

            def s_pass():
                """psum [32, OV] = sum over local i of c * u_hat."""
                sps = pss_pool.tile([32, OV], F32, tag="s0")
                for qb in range(NQ // 4):
                    t = tmp_pool.tile([128, 4 * OV], BF16, tag="tp")
                    if not do_smult:
                        nc.vector.memset(t[:], 0.01)
                    for qq in range(4 if do_smult else 0):
                        q = qb * 4 + qq
                        nc.vector.tensor_tensor(
                            t[:, qq * OV:(qq + 1) * OV].rearrange(
                                "p (o v) -> p o v", o=O),
                            U1[:, q * OV:(q + 1) * OV].rearrange(
                                "p (o v) -> p o v", o=O),
                            _bcast_inner(ct[:, q * O:(q + 1) * O], V),
                            op=ALU.mult)
                    first = qb == 0
                    last = qb == NQ // 4 - 1
                    for qq in range(4):
                        for h in range(2):
                            nc.tensor.matmul(
                                sps[:, h * 512:(h + 1) * 512],
                                sel_t[:],
                                t[:, qq * OV + h * 512:qq * OV + (h + 1) * 512],
                                start=(first and qq == 0),
                                stop=(last and qq == 3),
                            )
                return sps

            # ---------- iteration 0 ----------
            nc.vector.tensor_copy(sg[:], s0ps[:])
            if phase_a_only:
                nc.sync.dma_start(y.ap(), sg[:])
            else:
                for _rep in range(repeat):
                    allreduce_s(sg[:])
                    squash_to_w(1.0 / O)
                    a_pass(a0f)

                    # ---------- iteration 1 ----------
                    softmax(a0f, scratch=a1f)
                    s1 = s_pass()
                    nc.vector.tensor_copy(sg[:], s1[:])
                    allreduce_s(sg[:])
                    squash_to_w(1.0)
                    a_pass(a1f)
                    nc.vector.tensor_tensor(a1f[:], a1f[:], a0f[:], op=ALU.add)

                    # ---------- iteration 2 ----------
                    softmax(a1f, scratch=a0f)
                    s2 = s_pass()
                    nc.vector.tensor_copy(sg[:], s2[:])
                    allreduce_s(sg[:])
                    squash_to_w(1.0)
                    if _rep == 0:
                        nc.sync.dma_start(y.ap(), wv[:])
                    # feed sg back so repeats aren't dead code
                    nc.vector.tensor_copy(sg[:], wv[:])

    split_multi_waits(nc)
    return nc


def _prep_core(x, W, k):
    xk = x[:, k * IL:(k + 1) * IL, :]                    # [B, IL, D]
    Wk = W[:, k * IL:(k + 1) * IL, :, :]                 # [O, IL, V, D]
    xt = np.ascontiguousarray(xk.transpose(1, 2, 0))     # [IL, D, B]
    Wt = np.ascontiguousarray(Wk.transpose(1, 3, 0, 2))  # [IL, D, O, V]
    wa = Wt.reshape(NCH, 8, D, OV).reshape(NCH, 128, OV)
    x2 = xt.reshape(NCH, 8, D, B).reshape(NCH, 128, B)
    xtc = xt.reshape(NCH, 8, D, B)
    xqm = np.zeros((NCH, 128, 256), np.float32)
    for jj in range(2):
        for j in range(4):
            r = 4 * jj + j
            xqm[:, 64 * jj + 16 * j:64 * jj + 16 * (j + 1),
                128 * jj + 32 * j:128 * jj + 32 * (j + 1)] = xtc[:, r]
    return {"wa": np.ascontiguousarray(wa), "xq": xqm,
            "x2": np.ascontiguousarray(x2)}


def _sel_np():
    E = np.zeros((128, 32), np.float32)
    for j in range(4):
        E[32 * j + np.arange(32), np.arange(32)] = 1.0
    return E


def kernel(x: np.ndarray, W: np.ndarray, _trace=False) -> np.ndarray:
    x = np.asarray(x, np.float32)
    W = np.asarray(W, np.float32)
    if "nc" not in _CACHE:
        _CACHE["nc"] = _build_nc()
    nc = _CACHE["nc"]

    import ml_dtypes
    sel = _sel_np().astype(ml_dtypes.bfloat16)
    in_maps = []
    for k in range(NCORES):
        m = _prep_core(x, W, k)
        m["sel"] = sel
        in_maps.append(m)
    res = run_bass_kernel_spmd(nc, in_maps, list(range(NCORES)),
                               trace=_trace)
    if _trace:
        _CACHE["last_results"] = res
    out = res.results[0]["y"].reshape(B, O, V)
    return np.ascontiguousarray(out.astype(np.float32))

